# revision 51
# baseline (speedup 1.0000x reference)
"""Multi-head causal attention (B=2, S=4096, D=1024, H=16) on 8 TRN2 NeuronCores.

Sharding: head-parallel. Core c computes heads 2c, 2c+1 (128 of the 1024
projection columns) for both batches:
  - QKV column-parallel: each core gets Wq/Wk/Wv[:, c*128:(c+1)*128]
  - attention for its 2 heads over all tokens (causal, flash-free: full
    score rows, streamed in 512-query chunks, keys on PSUM partitions;
    the two heads' score matmuls are packed into PE row-groups 0-63 /
    64-127 via tile_position and run concurrently)
  - out-proj row-parallel: partial_out = ctx_c @ Wo[c*128:(c+1)*128, :]
  - host sums the 8 partials and adds bo.

x is transposed on the host (xT = x.reshape(T, D).T) because every matmul
on the PE contracts over the partition axis; this avoids all on-chip
transposes.

Layouts on-chip (per core):
  qT, kT:  [128, T]  rows 0:64 head0, 64:128 head1 (transposed projections)
  vA:      [128, T/128, 130]  per key-tile [v_h0 | ones | v_h1 | ones]
           (the ones column makes the ctx matmul also produce the softmax
           denominator as output row 64)
  scoresT: PSUM [128 keys, 2x512 queries (both heads)] -> exp on ACT -> SBUF
  ctxT:    [128, T]  accumulated in PSUM [65, 512] per (head, query chunk);
           softmax denominators ride along as output row 64
"""

from contextlib import ExitStack

import numpy as np

import concourse.bass as bass
import concourse.tile as tile
from concourse import bacc, mybir
from concourse.bass_utils import run_bass_kernel_spmd

F32 = mybir.dt.float32
BF16 = mybir.dt.bfloat16
P = 128
AF = mybir.ActivationFunctionType

N_CORES = 8
B_FULL, S_FULL, D_FULL, H_FULL = 2, 4096, 1024, 16
DH = 64
CW = 128  # projection columns per core (2 heads * 64)


def build_program(S=S_FULL, B=B_FULL, D=D_FULL):
    """Build the per-core Bass program (same program on all 8 cores)."""
    T = B * S
    KC = D // P            # contraction chunks for the projections
    IC = min(512, S)       # query-chunk width (paired-head layout)
    NJ = S // P            # key tiles per batch
    NIC = S // IC          # query chunks per batch
    WN = min(512, T)       # QKV token window

    nc = bacc.Bacc("TRN2", target_bir_lowering=False, debug=False,
                   num_devices=N_CORES)

    xT = nc.dram_tensor("xT", [D, T], BF16, kind="ExternalInput").ap()
    wq = nc.dram_tensor("wq", [P, D // P, CW], BF16, kind="ExternalInput").ap()
    wk = nc.dram_tensor("wk", [P, D // P, CW], BF16, kind="ExternalInput").ap()
    wv = nc.dram_tensor("wv", [P, D // P, CW], BF16, kind="ExternalInput").ap()
    wo = nc.dram_tensor("wo", [CW, D], BF16, kind="ExternalInput").ap()
    mask = nc.dram_tensor("mask", [P, P], BF16, kind="ExternalInput").ap()
    out = nc.dram_tensor("out", [T, D], BF16, kind="ExternalOutput").ap()
    # bounce rows for broadcasting 1/sum across partitions via DMA
    nrm = nc.dram_tensor("nrm_scratch", [B * NIC * 2, IC], F32).ap()

    with tile.TileContext(nc) as tc, ExitStack() as ctx:
        singles = ctx.enter_context(tc.tile_pool(name="singles", bufs=1))
        qT = singles.tile([P, T], BF16, name="qT")
        kT = singles.tile([P, T], BF16, name="kT")
        vA = singles.tile([P, B * NJ, 130], BF16, name="vA")
        cT = singles.tile([P, T], BF16, name="cT")
        wq_s = singles.tile([P, KC, CW], BF16, name="wq_s")
        wk_s = singles.tile([P, KC, CW], BF16, name="wk_s")
        wv_s = singles.tile([P, KC, CW], BF16, name="wv_s")
        wo_s = singles.tile([CW, D], BF16, name="wo_s")
        mask_s = singles.tile([P, P], BF16, name="mask_s")
        ones65 = singles.tile([65, 64], F32, name="ones65")

        nc.sync.dma_start(out=wq_s, in_=wq)
        nc.sync.dma_start(out=wk_s, in_=wk)
        nc.sync.dma_start(out=wv_s, in_=wv)
        nc.vector.memset(ones65[64:65, :], 1.0)
        nc.vector.memset(vA[:, :, 64:65], 1.0)
        nc.vector.memset(vA[:, :, 129:130], 1.0)

        xw_pool = ctx.enter_context(tc.tile_pool(name="xw_pool", bufs=3))
        # PSUM budget (8 banks): sc 2x2 + cx 2x1 + shared-small 2x1 = 8
        sm_ps = ctx.enter_context(
            tc.tile_pool(name="sm_ps", bufs=2, space=bass.MemorySpace.PSUM))
        sc_ps = ctx.enter_context(
            tc.tile_pool(name="sc_ps", bufs=2, space=bass.MemorySpace.PSUM))
        cx_ps = ctx.enter_context(
            tc.tile_pool(name="cx_ps", bufs=2, space=bass.MemorySpace.PSUM))
        exp_sb = ctx.enter_context(tc.tile_pool(name="exp_sb", bufs=4))
        st_sb = ctx.enter_context(tc.tile_pool(name="st_sb", bufs=2))
        nrm_sb = ctx.enter_context(tc.tile_pool(name="nrm_sb", bufs=2))
        ob_sb = ctx.enter_context(tc.tile_pool(name="ob_sb", bufs=2))

        def emit_qkv_window(w):
            xw = xw_pool.tile([P, KC, WN], BF16, name="xw", tag="xw")
            for kc in range(KC):
                nc.sync.dma_start(
                    out=xw[:, kc, :],
                    in_=xT[kc * P:(kc + 1) * P, w * WN:(w + 1) * WN])
            q_ps = sm_ps.tile([P, WN], F32, name="q_ps", tag="sm")
            for kc in range(KC):
                nc.tensor.matmul(q_ps, wq_s[:, kc, :], xw[:, kc, :],
                                 start=(kc == 0), stop=(kc == KC - 1))
            nc.vector.tensor_copy(qT[:, w * WN:(w + 1) * WN], q_ps)
            k_ps = sm_ps.tile([P, WN], F32, name="k_ps", tag="sm")
            for kc in range(KC):
                nc.tensor.matmul(k_ps, wk_s[:, kc, :], xw[:, kc, :],
                                 start=(kc == 0), stop=(kc == KC - 1))
            nc.vector.tensor_copy(kT[:, w * WN:(w + 1) * WN], k_ps)
            for st in range(WN // P):
                jt = (w * WN) // P + st  # global token tile
                vp = sm_ps.tile([P, CW], F32, name="vp", tag="sm")
                for kc in range(KC):
                    nc.tensor.matmul(vp, xw[:, kc, st * P:(st + 1) * P],
                                     wv_s[:, kc, :],
                                     start=(kc == 0), stop=(kc == KC - 1))
                nc.vector.tensor_copy(vA[:, jt, 0:64], vp[:, 0:64])
                nc.vector.tensor_copy(vA[:, jt, 65:129], vp[:, 64:128])

        def emit_attn_chunk(b, icn, tail=False):
            gi0 = b * S + icn * IC  # global query start
            njt = (icn + 1) * (IC // P)
            # one cx tile [65, IC<=512] (1 bank) per head
            cxs = [cx_ps.tile([65, IC], F32, name="cx", tag="cx")
                   for _ in range(2)]
            seen = [set(), set()]

            def emit_ctx(jt, ex, cpieces):
                for (h, a, bnd, stp) in cpieces:
                    strt = jt == 0 and 0 not in seen[h]
                    seen[h].add(0)
                    nc.tensor.matmul(
                        cxs[h][:, a:bnd],
                        vA[:, b * NJ + jt, h * 65:(h + 1) * 65],
                        ex[:, h * IC + a:h * IC + bnd],
                        start=strt, stop=stp)

            pend = None
            for jt in range(njt):
                il0 = max(0, jt * P - icn * IC)
                gj0 = b * S + jt * P
                # paired scores: h0 -> cols [0:IC), h1 -> cols [IC:2IC)
                # of one psum tile; tile_position row-split (0,0)/(64,0)
                # lets the two matmuls run concurrently on the PE
                sc = sc_ps.tile([P, 2 * IC], F32, name="sc", tag="sc")
                for h in range(2):
                    hp = h * 64
                    nc.tensor.matmul(
                        sc[:, h * IC + il0:(h + 1) * IC],
                        kT[hp:hp + 64, gj0:gj0 + P],
                        qT[hp:hp + 64, gi0 + il0:gi0 + IC],
                        start=True, stop=True)
                ex = exp_sb.tile([P, 2 * IC], BF16, name="ex", tag="ex")
                if il0 == 0:
                    nc.scalar.activation(ex[:, 0:2 * IC], sc[:, 0:2 * IC],
                                         AF.Exp, scale=0.125)
                else:
                    # diagonal tiles: the scores matmuls only wrote
                    # [il0, IC) per head, so exp each head's range
                    for h in range(2):
                        nc.scalar.activation(
                            ex[:, h * IC + il0:(h + 1) * IC],
                            sc[:, h * IC + il0:(h + 1) * IC],
                            AF.Exp, scale=0.125)
                diag = jt * P >= icn * IC
                if diag:  # mask both heads' diagonal blocks
                    nc.vector.tensor_mul(ex[:, il0:il0 + P],
                                         ex[:, il0:il0 + P], mask_s)
                    nc.vector.tensor_mul(ex[:, IC + il0:IC + il0 + P],
                                         ex[:, IC + il0:IC + il0 + P],
                                         mask_s)
                # ctx pieces per head; stop on the final diagonal piece
                cpieces = []
                for h in range(2):
                    if diag:
                        cpieces.append((h, il0, il0 + P, il0 + P == IC))
                        if il0 + P < IC:
                            cpieces.append((h, il0 + P, IC, False))
                    else:
                        cpieces.append((h, il0, IC, False))
                # lag-1 software pipeline
                if pend is not None:
                    emit_ctx(*pend)
                pend = (jt, ex, cpieces)
            emit_ctx(*pend)
            # evacuate + normalize both heads (h1 first: its
            # chain has an extra DMA; overlap it under h0's)
            for h in (1, 0):
                stage = st_sb.tile([65, IC], F32, name="stage", tag="stage")
                nc.vector.tensor_copy(stage, cxs[h])
                nc.vector.reciprocal(stage[64:65, :], stage[64:65, :])
                if tail:
                    # PE is idle at the kernel tail: broadcast 1/sum
                    # across partitions with a K=1 matmul instead of the
                    # higher-latency DRAM-bounce DMA pair
                    rb = sm_ps.tile([64, IC], F32, name="rbp", tag="sm")
                    nc.tensor.matmul(rb, ones65[64:65, :],
                                     stage[64:65, :], start=True, stop=True)
                else:
                    ni = (b * NIC + icn) * 2 + h
                    nc.sync.dma_start(out=nrm[ni:ni + 1, :],
                                      in_=stage[64:65, :])
                    src = nrm[ni:ni + 1, :]
                    bc = bass.AP(tensor=src.tensor, offset=src.offset,
                                 ap=[[0, 64], src.ap[-1]])
                    rb = nrm_sb.tile([64, IC], F32, name="rb", tag="rb")
                    nc.sync.dma_start(out=rb, in_=bc)
                if h == 0:
                    nc.vector.tensor_mul(cT[0:64, gi0:gi0 + IC],
                                         stage[0:64, :], rb)
                else:
                    tmp = nrm_sb.tile([64, IC], BF16, name="tmp", tag="tmp")
                    nc.vector.tensor_mul(tmp, stage[0:64, :], rb)
                    nc.sync.dma_start(out=cT[64:128, gi0:gi0 + IC], in_=tmp)
            # out-projection for this query chunk
            for st in range(IC // P):
                s0 = gi0 + st * P
                ob = ob_sb.tile([P, D], BF16, name="ob", tag="ob")
                for nn in range(D // 512):
                    op = sm_ps.tile([P, 512], F32, name="op", tag="sm")
                    nc.tensor.matmul(op, cT[:, s0:s0 + P],
                                     wo_s[:, nn * 512:(nn + 1) * 512],
                                     start=True, stop=True)
                    if tail and nn % 2 == 0:
                        # ACT is idle at the kernel tail; split the psum
                        # evacuation across both engines
                        nc.scalar.copy(ob[:, nn * 512:(nn + 1) * 512], op)
                    else:
                        nc.vector.tensor_copy(ob[:, nn * 512:(nn + 1) * 512],
                                              op)
                nc.sync.dma_start(out=out[s0:s0 + P, :], in_=ob)

        # Emission: fully pipelined. Window w covers tokens
        # [w*WN,(w+1)*WN); chunk (b, icn) only needs windows covering
        # tokens < b*S + (icn+1)*IC. Emitting windows two ahead of the
        # chunk that needs them keeps their DMA off the critical path
        # while attention (ACT-bound) overlaps the projection matmuls.
        nwin = T // WN
        state = {"emitted": 0}

        def need(upto):
            while state["emitted"] < min(upto, nwin):
                emit_qkv_window(state["emitted"])
                if state["emitted"] == 0:
                    # deferred: not needed before the first attention chunk
                    nc.sync.dma_start(out=wo_s, in_=wo)
                    nc.sync.dma_start(out=mask_s, in_=mask)
                state["emitted"] += 1

        for b in range(B):
            for icn in range(NIC):
                need((b * S + (icn + 1) * IC + WN - 1) // WN + 2)
                emit_attn_chunk(b, icn,
                                tail=(b == B - 1 and icn == NIC - 1))
        need(nwin)

    nc.compile()
    return nc


def _warrange(w, bf16):
    # [D, CW] -> [P, D//P, CW] contiguous (the SBUF layout, so the DMA is
    # a single contiguous copy instead of 256B strided pieces)
    D, CW_ = w.shape
    return np.ascontiguousarray(
        w.reshape(D // P, P, CW_).transpose(1, 0, 2)).astype(bf16)


def make_in_maps(x, Wq, Wk, Wv, Wo):
    import ml_dtypes
    bf16 = ml_dtypes.bfloat16
    B, S, D = x.shape
    xT = np.ascontiguousarray(x.reshape(B * S, D).T).astype(bf16)
    mask = np.triu(np.ones((P, P), dtype=bf16))
    in_maps = []
    for c in range(N_CORES):
        cs = slice(c * CW, (c + 1) * CW)
        in_maps.append({
            "xT": xT,
            "wq": _warrange(Wq[:, cs], bf16),
            "wk": _warrange(Wk[:, cs], bf16),
            "wv": _warrange(Wv[:, cs], bf16),
            "wo": np.ascontiguousarray(Wo[cs, :]).astype(bf16),
            "mask": mask,
        })
    return in_maps


_CACHED_NC = None


def kernel(x, Wq, Wk, Wv, Wo, bo, _trace=False):
    global _CACHED_NC
    x = np.asarray(x, dtype=np.float32)
    B, S, D = x.shape
    if _CACHED_NC is None:
        _CACHED_NC = build_program(S=S, B=B, D=D)
    nc = _CACHED_NC
    in_maps = make_in_maps(x, np.asarray(Wq), np.asarray(Wk),
                           np.asarray(Wv), np.asarray(Wo))
    res = None
    for attempt in range(3):
        try:
            res = run_bass_kernel_spmd(nc, in_maps, list(range(N_CORES)),
                                       trace=_trace)
            break
        except Exception:
            if attempt == 2:
                raise
    out = np.zeros((B * S, D), dtype=np.float32)
    for c in range(N_CORES):
        out += res.results[c]["out"].astype(np.float32)
    out += np.asarray(bo, dtype=np.float32)[None, :]
    if _trace:
        kernel._last_result = res
    return out.reshape(B, S, D)

